# revision 23
# baseline (speedup 1.0000x reference)
"""Trainium2 Bass kernel for nn_Attention_55499567399068 (v3.1).

Episode-attention, data-parallel over batch across 8 NeuronCores
(32 episodes => 256 (b, n) pairs per core), 32 superblocks of 8 pairs.

v3.1 structure:
  - Host pre-casts inputs to bf16 and pre-transposes Xq/Xk/Xv, so the device
    does plain HWDGE loads only (no cast DMAs, no input transpose DMAs).
  - softmax row-constant folding:
      scores ~ Xq M Xk^T + 1 v^T,  M = Wq^T Wk/sqrt(d),  v = Xk (Wk^T bq)/sqrt(d)
    and v folds further into the q-side projection: Y = Xq M + 1 wkb^T, i.e.
    a per-partition bias added during the Y^T PSUM->SBUF copies. The k-side
    projection disappears entirely.
  - scores computed in 2-pair dense [128,128] blocks (cross-pair entries are
    garbage, ignored by the staggered exp).
  - reduce_att algebra: P = Xv W1v^T (W1v = Wr1 Wv), hidT = P^T A^T per pair,
    prelu(hidT + b1), w = prelu^T wr2 + br2, g = A^T w, z = Xv^T g,
    out = Wv z + bv sum(w)  (A row sums = 1 absorb all bias plumbing).
  - Prelu (parametric_relu) instead of Lrelu: same activation table as
    Exp/Copy/Identity => no per-superblock act-table reloads.
  - final projection as out[pr,h] = sum_d z[d,pr] WvT[d,h]: zS is the
    8-column stationary; output lands in natural [8,512] rows -> plain store.
  - PSUM budget (8 banks): ytag x4, s_all x1, (P+small accumulators) x2,
    (hid + outN at partitions 64:72 via tile_position) x1.
"""

import sys

sys.path.insert(0, "/opt/trn_rl_repo")

import ml_dtypes
import numpy as np

import concourse.bass as bass
import concourse.tile as tile
from concourse import bacc, mybir
from concourse.bass_utils import run_bass_kernel_spmd

F32 = mybir.dt.float32
BF16 = mybir.dt.bfloat16
BF16_NP = ml_dtypes.bfloat16

BS, NWAY, NSHOT, D = 256, 8, 64, 512
NCORES = 8
BS_SH = BS // NCORES
NPAIR = BS_SH * NWAY
SUPER = 8
NSB = NPAIR // SUPER
ROWS_SB = SUPER * NSHOT
LEAK = 0.01
AT = mybir.ActivationFunctionType
ALU = mybir.AluOpType

BR2_VAL = [0.0]
Z_VIA_XBAR = [True]
DBG_RELU = [False]
USE_LRELU = [False]
STAGE = [99]

# offsets inside the shared P+smalls PSUM bank (f32 columns)
PCOL = 256  # P occupies cols 0:256
WOFF, GOFF, SWOFF, ZOFF = 256, 260, 264, 288


def build_nc(repeat=1, n_sb=NSB):
    nc = bacc.Bacc("TRN2", target_bir_lowering=False)

    # inputs, host-prearranged: [n_sb*128, 2048] bf16 (see prep_in_maps)
    xqT_d = nc.dram_tensor("xqT", [NSB * 128, 2048], BF16, kind="ExternalInput")
    xkT_d = nc.dram_tensor("xkT", [NSB * 128, 2048], BF16, kind="ExternalInput")
    xvT_d = nc.dram_tensor("xvT", [NSB * 128, 2048], BF16, kind="ExternalInput")
    xv_d = nc.dram_tensor("xv", [NSB * 128, 2048], BF16, kind="ExternalInput")
    # constants
    mT_d = nc.dram_tensor("mT", [D, D], BF16, kind="ExternalInput")
    wvT_d = nc.dram_tensor("wvT", [D, D], BF16, kind="ExternalInput")
    w1vT_d = nc.dram_tensor("w1vT", [D, 64], BF16, kind="ExternalInput")
    wkb_d = nc.dram_tensor("wkb", [128, 4], F32, kind="ExternalInput")
    b1c_d = nc.dram_tensor("b1c", [128, 1], F32, kind="ExternalInput")
    wr2s_d = nc.dram_tensor("wr2s", [128, 1], BF16, kind="ExternalInput")
    bvr_d = nc.dram_tensor("bvr", [1, D], BF16, kind="ExternalInput")
    ones_d = nc.dram_tensor("ones", [128, 1], BF16, kind="ExternalInput")
    out_d = nc.dram_tensor("out", [NPAIR, D], F32, kind="ExternalOutput")

    with tile.TileContext(nc) as tc:
        import contextlib

        ctx = contextlib.ExitStack()
        with ctx:
            const_pool = ctx.enter_context(tc.tile_pool(name="const", bufs=1))
            ld_pool = ctx.enter_context(tc.tile_pool(name="loads", bufs=4))
            work_pool = ctx.enter_context(tc.tile_pool(name="work", bufs=3))
            psY = ctx.enter_context(tc.tile_pool(name="psY", bufs=3, space="PSUM"))
            psS = ctx.enter_context(tc.tile_pool(name="psS", bufs=1, space="PSUM"))
            psP = ctx.enter_context(tc.tile_pool(name="psP", bufs=2, space="PSUM"))
            psH = ctx.enter_context(tc.tile_pool(name="psH", bufs=1, space="PSUM"))
            psZ = ctx.enter_context(tc.tile_pool(name="psZ", bufs=1, space="PSUM"))

            mT = const_pool.tile([128, 4 * D], BF16, tag="mT")
            wvT = const_pool.tile([128, 4 * D], BF16, tag="wvT")
            w1vT = const_pool.tile([128, 4 * 64], BF16, tag="w1vT")
            wkb = const_pool.tile([128, 4], F32, tag="wkb")
            b1c2 = const_pool.tile([128, 1], F32, tag="b1c2")
            wr2s = const_pool.tile([128, 1], BF16, tag="wr2s")
            bvr = const_pool.tile([1, D], BF16, tag="bvr")
            ones = const_pool.tile([128, 1], BF16, tag="ones")

            def load_consts():
                nc.sync.dma_start(
                    mT[:].rearrange("p (dc e) -> p dc e", dc=4),
                    mT_d[:, :].rearrange("(dc p) e -> p dc e", p=128),
                )
                nc.sync.dma_start(
                    wvT[:].rearrange("p (dc h) -> p dc h", dc=4),
                    wvT_d[:, :].rearrange("(dc p) h -> p dc h", p=128),
                )
                nc.sync.dma_start(
                    w1vT[:].rearrange("p (dc m) -> p dc m", dc=4),
                    w1vT_d[:, :].rearrange("(dc p) m -> p dc m", p=128),
                )
                nc.sync.dma_start(wkb[:], wkb_d[:, :])
                nc.sync.dma_start(b1c2[:], b1c_d[:, :])
                nc.sync.dma_start(wr2s[:], wr2s_d[:, :])
                nc.sync.dma_start(bvr[:], bvr_d[:, :])
                nc.sync.dma_start(ones[:], ones_d[:, :])

            def emit_superblock(sb):
                # ---------- A: plain loads (host-prearranged layouts) -----
                xqt = ld_pool.tile([128, 2048], BF16, tag="xqt")
                xkt = ld_pool.tile([128, 2048], BF16, tag="xkt")
                xvt = ld_pool.tile([128, 2048], BF16, tag="xvt")
                xv = ld_pool.tile([128, 2048], BF16, tag="xv")
                nc.sync.dma_start(xqt[:], xqT_d[bass.ts(sb, 128), :])
                nc.sync.dma_start(xkt[:], xkT_d[bass.ts(sb, 128), :])
                nc.sync.dma_start(xvt[:], xvT_d[bass.ts(sb, 128), :])
                nc.sync.dma_start(xv[:], xv_d[bass.ts(sb, 128), :])

                def cut(stage):
                    if STAGE[0] <= stage:
                        outNs = work_pool.tile([8, 512], F32, tag="outNs")
                        nc.vector.memset(outNs[:], 0.0)
                        nc.gpsimd.dma_start(
                            out_d[bass.ts(sb, SUPER), :], outNs[:]
                        )
                        return True
                    return False

                if cut(1):
                    return
                # ---------- B: P = Xv W1v^T (+ small accumulators bank) ---
                psm = psP.tile([128, 512], F32, tag="psm")
                for kc in range(4):
                    for dc in range(4):
                        nc.tensor.matmul(
                            psm[:, kc * 64 : (kc + 1) * 64],
                            lhsT=xvt[
                                :, dc * 512 + kc * 128 : dc * 512 + (kc + 1) * 128
                            ],
                            rhs=w1vT[:, dc * 64 : (dc + 1) * 64],
                            start=(dc == 0),
                            stop=(dc == 3),
                        )
                # block-diag per 2-pair group: Psb2[:, kc, 0, :] even rows,
                # [:, kc, 1, :] odd rows; cross blocks stay zero.
                Psb2 = work_pool.tile([128, 512], BF16, tag="Psb2")
                if sb < work_pool.bufs:
                    nc.vector.memset(Psb2[:], 0.0)
                Pv = Psb2[:].rearrange("p (kc two m) -> p kc two m", kc=4, two=2)
                pmv = psm[:, 0:256].rearrange("p (kc m) -> p kc m", kc=4)
                nc.vector.tensor_copy(Pv[0:64, :, 0, :], pmv[0:64, :, :])
                nc.scalar.activation(Pv[64:128, :, 1, :], pmv[64:128, :, :], AT.Copy)

                if cut(2):
                    return
                # ---------- C: Y^T = M^T Xq^T + wkb (bias via copies) -----
                yTs = work_pool.tile([128, 2048], BF16, tag="yTs")
                for ec in range(4):
                    yps = psY.tile([128, 512], F32, tag="yt")
                    for dc in range(4):
                        nc.tensor.matmul(
                            yps[:],
                            lhsT=mT[:, dc * 512 + ec * 128 : dc * 512 + (ec + 1) * 128],
                            rhs=xqt[:, dc * 512 : (dc + 1) * 512],
                            start=(dc == 0),
                            stop=(dc == 3),
                        )
                    dst = yTs[:, ec * 512 : (ec + 1) * 512]
                    if ec % 2 == 0:
                        nc.vector.tensor_scalar(
                            dst, yps[:], wkb[:, ec : ec + 1], None, op0=ALU.add
                        )
                    else:
                        nc.scalar.activation(
                            dst, yps[:], AT.Identity, bias=wkb[:, ec : ec + 1]
                        )

                if cut(3):
                    return
                # ---------- D: scores, 2-pair dense blocks ----------------
                s_all = psS.tile([128, 512], F32, tag="sh")
                for g in range(4):
                    sl = slice(g * 128, (g + 1) * 128)
                    for ec in range(4):
                        nc.tensor.matmul(
                            s_all[:, sl],
                            lhsT=yTs[:, ec * 512 + g * 128 : ec * 512 + (g + 1) * 128],
                            rhs=xkt[:, ec * 512 + g * 128 : ec * 512 + (g + 1) * 128],
                            start=(ec == 0),
                            stop=(ec == 3),
                        )

                if cut(4):
                    return
                # ---------- E: softmax (staggered block-diag layout) ------
                # exp only the valid [64,64] blocks (strided 3D APs, one op
                # per parity) so the cross-pair zeros stay intact; row sums
                # via segmented tensor_reduce.
                e_pad = work_pool.tile([128, 512], BF16, tag="e_pad")
                if sb < work_pool.bufs:
                    nc.vector.memset(e_pad[:], 0.0)
                Zb = work_pool.tile([128, 8], F32, tag="Zb")
                ep4 = e_pad[:].rearrange("p (g two c) -> p g two c", two=2, c=64)
                sa4 = s_all[:].rearrange("p (g two c) -> p g two c", two=2, c=64)
                nc.scalar.activation(ep4[0:64, :, 0, :], sa4[0:64, :, 0, :], AT.Exp)
                nc.scalar.activation(
                    ep4[64:128, :, 1, :], sa4[64:128, :, 1, :], AT.Exp
                )
                rT = work_pool.tile([128, 8], F32, tag="rT")
                Zb3 = Zb[:].rearrange("p (g two) -> p g two", two=2)
                rT3 = rT[:].rearrange("p (g two) -> p g two", two=2)
                nc.vector.tensor_reduce(
                    Zb3[0:64, :, 0], ep4[0:64, :, 0, :],
                    mybir.AxisListType.X, ALU.add,
                )
                nc.vector.tensor_reduce(
                    Zb3[64:128, :, 1], ep4[64:128, :, 1, :],
                    mybir.AxisListType.X, ALU.add,
                )
                nc.vector.reciprocal(rT3[0:64, :, 0], Zb3[0:64, :, 0])
                nc.vector.reciprocal(rT3[64:128, :, 1], Zb3[64:128, :, 1])
                for pr in range(SUPER):
                    po = (pr % 2) * 64
                    nc.vector.tensor_scalar(
                        e_pad[po : po + 64, pr * 64 : (pr + 1) * 64],
                        e_pad[po : po + 64, pr * 64 : (pr + 1) * 64],
                        rT[po : po + 64, pr : pr + 1],
                        None,
                        op0=ALU.mult,
                    )

                if cut(5):
                    return
                # ---------- F: A^T blocks via one xbar transpose ----------
                etD = work_pool.tile([128, 4 * 128], BF16, tag="etD")
                nc.scalar.dma_start(
                    etD[:].rearrange("p (c i) -> p c i", c=4),
                    e_pad[:],
                    transpose=True,
                )

                if cut(6):
                    return
                # ---------- G: hidT, 2-pair block-diag, full-K matmuls ----
                # hid2 block g = [[hidT_even, 0], [0, hidT_odd]] (+ prelu
                # turns the 0 blocks into prelu(b1), corrected via br2).
                hid2 = psH.tile([128, 512], F32, tag="ho")
                for g in range(4):
                    nc.tensor.matmul(
                        hid2[:, g * 128 : (g + 1) * 128],
                        lhsT=Psb2[:, g * 128 : (g + 1) * 128],
                        rhs=etD[:, g * 128 : (g + 1) * 128],
                        start=True,
                        stop=True,
                    )
                ys2 = work_pool.tile([128, 512], BF16, tag="ys2")
                act_fn = (
                    AT.Relu if DBG_RELU[0]
                    else (AT.Lrelu if USE_LRELU[0] else AT.Prelu)
                )
                nc.scalar.activation(
                    ys2[:], hid2[:], act_fn, bias=b1c2[:], alpha=LEAK
                )

                if cut(7):
                    return
                # ---------- H: w (2-pair), whr, Sw ------------------------
                for g in range(4):
                    nc.tensor.matmul(
                        psm[:, WOFF + g : WOFF + g + 1],
                        lhsT=ys2[:, g * 128 : (g + 1) * 128],
                        rhs=wr2s[:],
                        start=True,
                        stop=True,
                    )
                whr = work_pool.tile([128, 4], BF16, tag="whr")
                nc.vector.tensor_scalar(
                    whr[:], psm[:, WOFF : WOFF + 4], float(BR2_VAL[0]), None,
                    op0=ALU.add,
                )
                # Sw via block-diag wG so the matmul operands stay at base 0
                wG = work_pool.tile([128, 8], BF16, tag="wG")
                if sb < work_pool.bufs:
                    nc.vector.memset(wG[:], 0.0)
                wGv = wG[:].rearrange("p (g two) -> p g two", two=2)
                nc.vector.tensor_copy(wGv[0:64, :, 0], whr[0:64, :])
                nc.vector.tensor_copy(wGv[64:128, :, 1], whr[64:128, :])
                nc.tensor.matmul(
                    psm[0:1, SWOFF : SWOFF + 8],
                    lhsT=ones[:], rhs=wG[:], start=True, stop=True,
                )
                swS = work_pool.tile([1, 8], BF16, tag="swS")
                nc.vector.tensor_copy(swS[:], psm[0:1, SWOFF : SWOFF + 8])

                if cut(8):
                    return
                # ---------- I: g = A^T w (2-pair blocks) ------------------
                for g in range(4):
                    nc.tensor.matmul(
                        psm[:, GOFF + g : GOFF + g + 1],
                        lhsT=e_pad[:, g * 128 : (g + 1) * 128],
                        rhs=whr[:, g : g + 1],
                        start=True,
                        stop=True,
                    )
                gG = work_pool.tile([128, 32], BF16, tag="gG")
                g3 = gG[:].rearrange("p (r pr) -> p r pr", r=4)
                nc.vector.memset(gG[:], 0.0)
                for r in range(4):
                    nc.vector.tensor_copy(
                        g3[0:64, r, 2 * r : 2 * r + 1],
                        psm[0:64, GOFF + r : GOFF + r + 1],
                    )
                    nc.vector.tensor_copy(
                        g3[64:128, r, 2 * r + 1 : 2 * r + 2],
                        psm[64:128, GOFF + r : GOFF + r + 1],
                    )

                if cut(9):
                    return
                if Z_VIA_XBAR[0]:
                    # ---------- J: z^T = G^T Xv --------------------------
                    # both operands k-on-partitions: block-diag gG (8-col
                    # stationary, ~free LDW) + natural-layout Xv row groups;
                    # a 16 KB xbar then yields the d-on-partitions zS.
                    zTp = psZ.tile([16, 512], F32, tag="zT")
                    zps_out = zTp
                    for r in range(4):
                        nc.tensor.matmul(
                            zTp[0:8, :],
                            lhsT=g3[:, r, :],
                            rhs=xv[:, r * 512 : (r + 1) * 512],
                            start=(r == 0),
                            stop=(r == 3),
                        )
                    zTs = work_pool.tile([16, 512], BF16, tag="zTs")
                    if sb < work_pool.bufs:
                        nc.vector.memset(zTs[:, :], 0.0)
                    nc.vector.tensor_copy(zTs[0:8, :], zTp[0:8, :])
                    zS = work_pool.tile([128, 64], BF16, tag="zS")
                    nc.scalar.dma_start(
                        zS[:].rearrange("p (c i) -> p c i", c=4),
                        zTs[:],
                        transpose=True,
                    )
                    zsl = [zS[:, dc * 16 : dc * 16 + 8] for dc in range(4)]
                else:
                    # ---------- J: z = Xv^T G ----------------------------
                    for dc in range(4):
                        for r in range(4):
                            nc.tensor.matmul(
                                psm[:, ZOFF + dc * 8 : ZOFF + 8 + dc * 8],
                                lhsT=xv[
                                    :, r * 512 + dc * 128 : r * 512 + (dc + 1) * 128
                                ],
                                rhs=g3[:, r, :],
                                start=(r == 0),
                                stop=(r == 3),
                            )
                    zS = work_pool.tile([128, 32], BF16, tag="zS")
                    nc.vector.tensor_copy(zS[:], psm[:, ZOFF : ZOFF + 32])
                    zsl = [zS[:, dc * 8 : (dc + 1) * 8] for dc in range(4)]
                    zps_out = psZ.tile([16, 512], F32, tag="zT")

                if cut(10):
                    return
                # ---------- K: outN = z^T Wv^T + Sw (x) bv ----------------
                outp = zps_out[0:8, :]
                for dc in range(4):
                    nc.tensor.matmul(
                        outp,
                        lhsT=zsl[dc],
                        rhs=wvT[:, dc * 512 : (dc + 1) * 512],
                        start=(dc == 0),
                        stop=False,
                    )
                nc.tensor.matmul(
                    outp, lhsT=swS[:], rhs=bvr[:], start=False, stop=True,
                )
                outNs = work_pool.tile([8, 512], F32, tag="outNs")
                nc.scalar.activation(outNs[:], outp, AT.Copy)
                nc.gpsimd.dma_start(out_d[bass.ts(sb, SUPER), :], outNs[:])

            def body(_iv=None):
                load_consts()
                for sb in range(n_sb):
                    emit_superblock(sb)

            if repeat == 1:
                body()
            else:
                with tc.For_i(0, repeat, 1) as _iv:
                    body(_iv)

    nc.compile()
    return nc


def _prearrange_T(X):
    """[16384, 512] -> transposed load layout [sb*128+p, dc*512+r]."""
    return (
        X.T.reshape(4, 128, NSB, 512).transpose(2, 1, 0, 3).reshape(NSB * 128, 2048)
    ).copy()


def _prearrange_N(X):
    """[16384, 512] -> natural load layout [sb*128+p, r*512+d]."""
    return (
        X.reshape(NSB, 4, 128, 512).transpose(0, 2, 1, 3).reshape(NSB * 128, 2048)
    ).copy()


def prep_in_maps(query, key, value, Wq, bq, Wk, bk, Wv, bv, Wr1, br1, Wr2, br2):
    s = np.float32(1.0 / np.sqrt(np.float32(D)))
    # scores = Xq (Wq^T Wk * s) Xk^T + 1 (Xk Wk^T bq * s)^T  (+ row consts,
    # which softmax ignores); the second term folds into Y as "+ 1 wkb^T".
    M = (Wq.T.astype(np.float32) @ Wk.astype(np.float32)) * s  # [d, e]
    mT = M.astype(BF16_NP).copy()
    wkb_full = (Wk.T.astype(np.float32) @ bq.astype(np.float32)) * s  # [e]
    wkb = wkb_full.astype(np.float32).reshape(4, 128).T.copy()  # [128, 4]
    wvT = Wv.T.astype(BF16_NP).copy()
    w1vT = (Wr1 @ Wv).T.astype(BF16_NP).copy()
    b1 = (br1 + Wr1 @ bv).astype(np.float32)
    b1c = np.concatenate([b1, b1]).reshape(128, 1).copy()
    wr2s = np.concatenate([Wr2[0], Wr2[0]]).astype(BF16_NP).reshape(128, 1).copy()
    bvr = bv.astype(BF16_NP).reshape(1, D).copy()
    ones = np.ones((128, 1), dtype=BF16_NP)
    # the hid2 zero blocks turn into prelu(b1) and leak C into every w
    slope = 0.0 if DBG_RELU[0] else LEAK
    pb1 = np.where(b1 >= 0, b1, slope * b1).astype(np.float32)
    C = float(pb1 @ Wr2[0].astype(np.float32))
    BR2_VAL[0] = float(br2[0]) - C

    in_maps = []
    for c in range(NCORES):
        sl = slice(c * BS_SH, (c + 1) * BS_SH)
        Xq = np.ascontiguousarray(query[sl]).reshape(NPAIR * NSHOT, D).astype(BF16_NP)
        Xk = np.ascontiguousarray(key[sl]).reshape(NPAIR * NSHOT, D).astype(BF16_NP)
        Xv = np.ascontiguousarray(value[sl]).reshape(NPAIR * NSHOT, D).astype(BF16_NP)
        in_maps.append(
            {
                "xqT": _prearrange_T(Xq),
                "xkT": _prearrange_T(Xk),
                "xvT": _prearrange_T(Xv),
                "xv": _prearrange_N(Xv),
                "mT": mT,
                "wvT": wvT,
                "w1vT": w1vT,
                "wkb": wkb,
                "wr2s": wr2s,
                "b1c": b1c,
                "bvr": bvr,
                "ones": ones,
            }
        )
    return in_maps


_nc_cache = {}


def kernel(**inputs):
    in_maps = prep_in_maps(**{k: np.asarray(v) for k, v in inputs.items()})
    key = ("k", 1, BR2_VAL[0], Z_VIA_XBAR[0], DBG_RELU[0], USE_LRELU[0], STAGE[0])
    if key not in _nc_cache:
        _nc_cache[key] = build_nc(repeat=1)
    nc = _nc_cache[key]
    res = run_bass_kernel_spmd(nc, in_maps, core_ids=list(range(NCORES)))
    outs = [res.results[c]["out"].reshape(BS_SH, NWAY, D) for c in range(NCORES)]
    return np.concatenate(outs, axis=0).astype(np.float32)


# revision 24
# speedup vs baseline: 1.0248x; 1.0248x over previous
"""Trainium2 Bass kernel for nn_Attention_55499567399068 (v3.1).

Episode-attention, data-parallel over batch across 8 NeuronCores
(32 episodes => 256 (b, n) pairs per core), 32 superblocks of 8 pairs.

v3.1 structure:
  - Host pre-casts inputs to bf16 and pre-transposes Xq/Xk/Xv, so the device
    does plain HWDGE loads only (no cast DMAs, no input transpose DMAs).
  - softmax row-constant folding:
      scores ~ Xq M Xk^T + 1 v^T,  M = Wq^T Wk/sqrt(d),  v = Xk (Wk^T bq)/sqrt(d)
    and v folds further into the q-side projection: Y = Xq M + 1 wkb^T, i.e.
    a per-partition bias added during the Y^T PSUM->SBUF copies. The k-side
    projection disappears entirely.
  - scores computed in 2-pair dense [128,128] blocks (cross-pair entries are
    garbage, ignored by the staggered exp).
  - reduce_att algebra: P = Xv W1v^T (W1v = Wr1 Wv), hidT = P^T A^T per pair,
    prelu(hidT + b1), w = prelu^T wr2 + br2, g = A^T w, z = Xv^T g,
    out = Wv z + bv sum(w)  (A row sums = 1 absorb all bias plumbing).
  - Prelu (parametric_relu) instead of Lrelu: same activation table as
    Exp/Copy/Identity => no per-superblock act-table reloads.
  - final projection as out[pr,h] = sum_d z[d,pr] WvT[d,h]: zS is the
    8-column stationary; output lands in natural [8,512] rows -> plain store.
  - PSUM budget (8 banks): ytag x4, s_all x1, (P+small accumulators) x2,
    (hid + outN at partitions 64:72 via tile_position) x1.
"""

import sys

sys.path.insert(0, "/opt/trn_rl_repo")

import ml_dtypes
import numpy as np

import concourse.bass as bass
import concourse.tile as tile
from concourse import bacc, mybir
from concourse.bass_utils import run_bass_kernel_spmd

F32 = mybir.dt.float32
BF16 = mybir.dt.bfloat16
F8 = mybir.dt.float8e4
BF16_NP = ml_dtypes.bfloat16
F8_NP = ml_dtypes.float8_e4m3

BS, NWAY, NSHOT, D = 256, 8, 64, 512
NCORES = 8
BS_SH = BS // NCORES
NPAIR = BS_SH * NWAY
SUPER = 8
NSB = NPAIR // SUPER
ROWS_SB = SUPER * NSHOT
LEAK = 0.01
AT = mybir.ActivationFunctionType
ALU = mybir.AluOpType

BR2_VAL = [0.0]
Z_VIA_XBAR = [True]
DBG_RELU = [False]
USE_LRELU = [False]
STAGE = [99]
FP8_QK = [False]
FP8_SCALE = 64.0

# offsets inside the shared P+smalls PSUM bank (f32 columns)
PCOL = 256  # P occupies cols 0:256
WOFF, GOFF, SWOFF, ZOFF = 256, 260, 264, 288


def build_nc(repeat=1, n_sb=NSB):
    nc = bacc.Bacc("TRN2", target_bir_lowering=False)

    # inputs, host-prearranged: [n_sb*128, 2048] (see prep_in_maps)
    QKDT = F8 if FP8_QK[0] else BF16
    xqT_d = nc.dram_tensor("xqT", [NSB * 128, 2048], QKDT, kind="ExternalInput")
    xkT_d = nc.dram_tensor("xkT", [NSB * 128, 2048], QKDT, kind="ExternalInput")
    xvT_d = nc.dram_tensor("xvT", [NSB * 128, 2048], BF16, kind="ExternalInput")
    xv_d = nc.dram_tensor("xv", [NSB * 128, 2048], BF16, kind="ExternalInput")
    # constants
    mT_d = (
        nc.dram_tensor("mT8", [128, 2048], F8, kind="ExternalInput")
        if FP8_QK[0]
        else nc.dram_tensor("mT", [D, D], BF16, kind="ExternalInput")
    )
    wvT_d = nc.dram_tensor("wvT", [D, D], BF16, kind="ExternalInput")
    w1vT_d = nc.dram_tensor("w1vT", [D, 64], BF16, kind="ExternalInput")
    wkb_d = nc.dram_tensor("wkb", [128, 4], F32, kind="ExternalInput")
    b1c_d = nc.dram_tensor("b1c", [128, 1], F32, kind="ExternalInput")
    wr2s_d = nc.dram_tensor("wr2s", [128, 1], BF16, kind="ExternalInput")
    bvr_d = nc.dram_tensor("bvr", [1, D], BF16, kind="ExternalInput")
    ones_d = nc.dram_tensor("ones", [128, 1], BF16, kind="ExternalInput")
    out_d = nc.dram_tensor("out", [NPAIR, D], F32, kind="ExternalOutput")

    with tile.TileContext(nc) as tc:
        import contextlib

        ctx = contextlib.ExitStack()
        with ctx:
            const_pool = ctx.enter_context(tc.tile_pool(name="const", bufs=1))
            ld_pool = ctx.enter_context(tc.tile_pool(name="loads", bufs=4))
            work_pool = ctx.enter_context(tc.tile_pool(name="work", bufs=3))
            psY = ctx.enter_context(tc.tile_pool(name="psY", bufs=3, space="PSUM"))
            psS = ctx.enter_context(tc.tile_pool(name="psS", bufs=1, space="PSUM"))
            psP = ctx.enter_context(tc.tile_pool(name="psP", bufs=2, space="PSUM"))
            psH = ctx.enter_context(tc.tile_pool(name="psH", bufs=1, space="PSUM"))
            psZ = ctx.enter_context(tc.tile_pool(name="psZ", bufs=1, space="PSUM"))

            mT = const_pool.tile([128, 4 * D], QKDT, tag="mT")
            wvT = const_pool.tile([128, 4 * D], BF16, tag="wvT")
            w1vT = const_pool.tile([128, 4 * 64], BF16, tag="w1vT")
            wkb = const_pool.tile([128, 4], F32, tag="wkb")
            b1c2 = const_pool.tile([128, 1], F32, tag="b1c2")
            wr2s = const_pool.tile([128, 1], BF16, tag="wr2s")
            bvr = const_pool.tile([1, D], BF16, tag="bvr")
            ones = const_pool.tile([128, 1], BF16, tag="ones")

            def load_consts():
                if FP8_QK[0]:
                    nc.sync.dma_start(mT[:], mT_d[:, :])
                else:
                    nc.sync.dma_start(
                        mT[:].rearrange("p (dc e) -> p dc e", dc=4),
                        mT_d[:, :].rearrange("(dc p) e -> p dc e", p=128),
                    )
                nc.sync.dma_start(
                    wvT[:].rearrange("p (dc h) -> p dc h", dc=4),
                    wvT_d[:, :].rearrange("(dc p) h -> p dc h", p=128),
                )
                nc.sync.dma_start(
                    w1vT[:].rearrange("p (dc m) -> p dc m", dc=4),
                    w1vT_d[:, :].rearrange("(dc p) m -> p dc m", p=128),
                )
                nc.sync.dma_start(wkb[:], wkb_d[:, :])
                nc.sync.dma_start(b1c2[:], b1c_d[:, :])
                nc.sync.dma_start(wr2s[:], wr2s_d[:, :])
                nc.sync.dma_start(bvr[:], bvr_d[:, :])
                nc.sync.dma_start(ones[:], ones_d[:, :])

            def emit_superblock(sb):
                # ---------- A: plain loads (host-prearranged layouts) -----
                xqt = ld_pool.tile([128, 2048], QKDT, tag="xqt")
                xkt = ld_pool.tile([128, 2048], QKDT, tag="xkt")
                xvt = ld_pool.tile([128, 2048], BF16, tag="xvt")
                xv = ld_pool.tile([128, 2048], BF16, tag="xv")
                nc.sync.dma_start(xqt[:], xqT_d[bass.ts(sb, 128), :])
                nc.sync.dma_start(xkt[:], xkT_d[bass.ts(sb, 128), :])
                nc.sync.dma_start(xvt[:], xvT_d[bass.ts(sb, 128), :])
                nc.sync.dma_start(xv[:], xv_d[bass.ts(sb, 128), :])

                def cut(stage):
                    if STAGE[0] <= stage:
                        outNs = work_pool.tile([8, 512], F32, tag="outNs")
                        nc.vector.memset(outNs[:], 0.0)
                        nc.gpsimd.dma_start(
                            out_d[bass.ts(sb, SUPER), :], outNs[:]
                        )
                        return True
                    return False

                if cut(1):
                    return
                # ---------- B: P = Xv W1v^T (+ small accumulators bank) ---
                psm = psP.tile([128, 512], F32, tag="psm")
                for kc in range(4):
                    for dc in range(4):
                        nc.tensor.matmul(
                            psm[:, kc * 64 : (kc + 1) * 64],
                            lhsT=xvt[
                                :, dc * 512 + kc * 128 : dc * 512 + (kc + 1) * 128
                            ],
                            rhs=w1vT[:, dc * 64 : (dc + 1) * 64],
                            start=(dc == 0),
                            stop=(dc == 3),
                        )
                # block-diag per 2-pair group: Psb2[:, kc, 0, :] even rows,
                # [:, kc, 1, :] odd rows; cross blocks stay zero.
                Psb2 = work_pool.tile([128, 512], BF16, tag="Psb2")
                if sb < work_pool.bufs:
                    nc.vector.memset(Psb2[:], 0.0)
                Pv = Psb2[:].rearrange("p (kc two m) -> p kc two m", kc=4, two=2)
                pmv = psm[:, 0:256].rearrange("p (kc m) -> p kc m", kc=4)
                nc.vector.tensor_copy(Pv[0:64, :, 0, :], pmv[0:64, :, :])
                nc.scalar.activation(Pv[64:128, :, 1, :], pmv[64:128, :, :], AT.Copy)

                if cut(2):
                    return
                # ---------- C: Y^T = M^T Xq^T + wkb (bias via copies) -----
                yTs = work_pool.tile([128, 2048], QKDT, tag="yTs")
                mTv = mT[:].rearrange("p (d2 ko e) -> p d2 ko e", d2=2, ko=2)
                xqv = xqt[:].rearrange("p (d2 ko r) -> p d2 ko r", d2=2, ko=2)
                for ec in range(4):
                    yps = psY.tile([128, 512], F32, tag="yt")
                    if FP8_QK[0]:
                        for d2 in range(2):
                            nc.tensor.matmul(
                                yps[:],
                                lhsT=mTv[:, d2, :, ec * 128 : (ec + 1) * 128],
                                rhs=xqv[:, d2, :, :],
                                start=(d2 == 0),
                                stop=(d2 == 1),
                                perf_mode=mybir.MatmulPerfMode.DoubleRow,
                            )
                    else:
                        for dc in range(4):
                            nc.tensor.matmul(
                                yps[:],
                                lhsT=mT[
                                    :, dc * 512 + ec * 128 : dc * 512 + (ec + 1) * 128
                                ],
                                rhs=xqt[:, dc * 512 : (dc + 1) * 512],
                                start=(dc == 0),
                                stop=(dc == 3),
                            )
                    dst = yTs[:, ec * 512 : (ec + 1) * 512]
                    if ec % 2 == 0:
                        nc.vector.tensor_scalar(
                            dst, yps[:], wkb[:, ec : ec + 1], None, op0=ALU.add
                        )
                    else:
                        nc.scalar.activation(
                            dst, yps[:], AT.Identity, bias=wkb[:, ec : ec + 1]
                        )

                if cut(3):
                    return
                # ---------- D: scores, 2-pair dense blocks ----------------
                s_all = psS.tile([128, 512], F32, tag="sh")
                yv = yTs[:].rearrange("p (e2 ko r) -> p e2 ko r", e2=2, ko=2)
                xkv = xkt[:].rearrange("p (e2 ko r) -> p e2 ko r", e2=2, ko=2)
                for g in range(4):
                    sl = slice(g * 128, (g + 1) * 128)
                    if FP8_QK[0]:
                        for e2 in range(2):
                            nc.tensor.matmul(
                                s_all[:, sl],
                                lhsT=yv[:, e2, :, g * 128 : (g + 1) * 128],
                                rhs=xkv[:, e2, :, g * 128 : (g + 1) * 128],
                                start=(e2 == 0),
                                stop=(e2 == 1),
                                perf_mode=mybir.MatmulPerfMode.DoubleRow,
                            )
                    else:
                        for ec in range(4):
                            nc.tensor.matmul(
                                s_all[:, sl],
                                lhsT=yTs[
                                    :, ec * 512 + g * 128 : ec * 512 + (g + 1) * 128
                                ],
                                rhs=xkt[
                                    :, ec * 512 + g * 128 : ec * 512 + (g + 1) * 128
                                ],
                                start=(ec == 0),
                                stop=(ec == 3),
                            )

                if cut(4):
                    return
                # ---------- E: softmax (staggered block-diag layout) ------
                # exp only the valid [64,64] blocks (strided 3D APs, one op
                # per parity) so the cross-pair zeros stay intact; row sums
                # via segmented tensor_reduce.
                e_pad = work_pool.tile([128, 512], BF16, tag="e_pad")
                if sb < work_pool.bufs:
                    nc.vector.memset(e_pad[:], 0.0)
                Zb = work_pool.tile([128, 8], F32, tag="Zb")
                ep4 = e_pad[:].rearrange("p (g two c) -> p g two c", two=2, c=64)
                sa4 = s_all[:].rearrange("p (g two c) -> p g two c", two=2, c=64)
                esc = 1.0 / FP8_SCALE if FP8_QK[0] else 1.0
                nc.scalar.activation(
                    ep4[0:64, :, 0, :], sa4[0:64, :, 0, :], AT.Exp, scale=esc
                )
                nc.scalar.activation(
                    ep4[64:128, :, 1, :], sa4[64:128, :, 1, :], AT.Exp, scale=esc
                )
                rT = work_pool.tile([128, 8], F32, tag="rT")
                Zb3 = Zb[:].rearrange("p (g two) -> p g two", two=2)
                rT3 = rT[:].rearrange("p (g two) -> p g two", two=2)
                nc.vector.tensor_reduce(
                    Zb3[0:64, :, 0], ep4[0:64, :, 0, :],
                    mybir.AxisListType.X, ALU.add,
                )
                nc.vector.tensor_reduce(
                    Zb3[64:128, :, 1], ep4[64:128, :, 1, :],
                    mybir.AxisListType.X, ALU.add,
                )
                nc.vector.reciprocal(rT3[0:64, :, 0], Zb3[0:64, :, 0])
                nc.vector.reciprocal(rT3[64:128, :, 1], Zb3[64:128, :, 1])
                for pr in range(SUPER):
                    po = (pr % 2) * 64
                    nc.vector.tensor_scalar(
                        e_pad[po : po + 64, pr * 64 : (pr + 1) * 64],
                        e_pad[po : po + 64, pr * 64 : (pr + 1) * 64],
                        rT[po : po + 64, pr : pr + 1],
                        None,
                        op0=ALU.mult,
                    )

                if cut(5):
                    return
                # ---------- F: A^T blocks via one xbar transpose ----------
                etD = work_pool.tile([128, 4 * 128], BF16, tag="etD")
                nc.scalar.dma_start(
                    etD[:].rearrange("p (c i) -> p c i", c=4),
                    e_pad[:],
                    transpose=True,
                )

                if cut(6):
                    return
                # ---------- G: hidT, 2-pair block-diag, full-K matmuls ----
                # hid2 block g = [[hidT_even, 0], [0, hidT_odd]] (+ prelu
                # turns the 0 blocks into prelu(b1), corrected via br2).
                hid2 = psH.tile([128, 512], F32, tag="ho")
                for g in range(4):
                    nc.tensor.matmul(
                        hid2[:, g * 128 : (g + 1) * 128],
                        lhsT=Psb2[:, g * 128 : (g + 1) * 128],
                        rhs=etD[:, g * 128 : (g + 1) * 128],
                        start=True,
                        stop=True,
                    )
                ys2 = work_pool.tile([128, 512], BF16, tag="ys2")
                act_fn = (
                    AT.Relu if DBG_RELU[0]
                    else (AT.Lrelu if USE_LRELU[0] else AT.Prelu)
                )
                nc.scalar.activation(
                    ys2[:], hid2[:], act_fn, bias=b1c2[:], alpha=LEAK
                )

                if cut(7):
                    return
                # ---------- H: w (2-pair), whr, Sw ------------------------
                for g in range(4):
                    nc.tensor.matmul(
                        psm[:, WOFF + g : WOFF + g + 1],
                        lhsT=ys2[:, g * 128 : (g + 1) * 128],
                        rhs=wr2s[:],
                        start=True,
                        stop=True,
                    )
                whr = work_pool.tile([128, 4], BF16, tag="whr")
                nc.vector.tensor_scalar(
                    whr[:], psm[:, WOFF : WOFF + 4], float(BR2_VAL[0]), None,
                    op0=ALU.add,
                )
                # Sw via block-diag wG so the matmul operands stay at base 0
                wG = work_pool.tile([128, 8], BF16, tag="wG")
                if sb < work_pool.bufs:
                    nc.vector.memset(wG[:], 0.0)
                wGv = wG[:].rearrange("p (g two) -> p g two", two=2)
                nc.vector.tensor_copy(wGv[0:64, :, 0], whr[0:64, :])
                nc.vector.tensor_copy(wGv[64:128, :, 1], whr[64:128, :])
                nc.tensor.matmul(
                    psm[0:1, SWOFF : SWOFF + 8],
                    lhsT=ones[:], rhs=wG[:], start=True, stop=True,
                )
                swS = work_pool.tile([1, 8], BF16, tag="swS")
                nc.vector.tensor_copy(swS[:], psm[0:1, SWOFF : SWOFF + 8])

                if cut(8):
                    return
                # ---------- I: g = A^T w (2-pair blocks) ------------------
                for g in range(4):
                    nc.tensor.matmul(
                        psm[:, GOFF + g : GOFF + g + 1],
                        lhsT=e_pad[:, g * 128 : (g + 1) * 128],
                        rhs=whr[:, g : g + 1],
                        start=True,
                        stop=True,
                    )
                gG = work_pool.tile([128, 32], BF16, tag="gG")
                g3 = gG[:].rearrange("p (r pr) -> p r pr", r=4)
                nc.vector.memset(gG[:], 0.0)
                for r in range(4):
                    nc.vector.tensor_copy(
                        g3[0:64, r, 2 * r : 2 * r + 1],
                        psm[0:64, GOFF + r : GOFF + r + 1],
                    )
                    nc.vector.tensor_copy(
                        g3[64:128, r, 2 * r + 1 : 2 * r + 2],
                        psm[64:128, GOFF + r : GOFF + r + 1],
                    )

                if cut(9):
                    return
                if Z_VIA_XBAR[0]:
                    # ---------- J: z^T = G^T Xv --------------------------
                    # both operands k-on-partitions: block-diag gG (8-col
                    # stationary, ~free LDW) + natural-layout Xv row groups;
                    # a 16 KB xbar then yields the d-on-partitions zS.
                    zTp = psZ.tile([16, 512], F32, tag="zT")
                    zps_out = zTp
                    for r in range(4):
                        nc.tensor.matmul(
                            zTp[0:8, :],
                            lhsT=g3[:, r, :],
                            rhs=xv[:, r * 512 : (r + 1) * 512],
                            start=(r == 0),
                            stop=(r == 3),
                        )
                    zTs = work_pool.tile([16, 512], BF16, tag="zTs")
                    if sb < work_pool.bufs:
                        nc.vector.memset(zTs[:, :], 0.0)
                    nc.vector.tensor_copy(zTs[0:8, :], zTp[0:8, :])
                    zS = work_pool.tile([128, 64], BF16, tag="zS")
                    nc.scalar.dma_start(
                        zS[:].rearrange("p (c i) -> p c i", c=4),
                        zTs[:],
                        transpose=True,
                    )
                    zsl = [zS[:, dc * 16 : dc * 16 + 8] for dc in range(4)]
                else:
                    # ---------- J: z = Xv^T G ----------------------------
                    for dc in range(4):
                        for r in range(4):
                            nc.tensor.matmul(
                                psm[:, ZOFF + dc * 8 : ZOFF + 8 + dc * 8],
                                lhsT=xv[
                                    :, r * 512 + dc * 128 : r * 512 + (dc + 1) * 128
                                ],
                                rhs=g3[:, r, :],
                                start=(r == 0),
                                stop=(r == 3),
                            )
                    zS = work_pool.tile([128, 32], BF16, tag="zS")
                    nc.vector.tensor_copy(zS[:], psm[:, ZOFF : ZOFF + 32])
                    zsl = [zS[:, dc * 8 : (dc + 1) * 8] for dc in range(4)]
                    zps_out = psZ.tile([16, 512], F32, tag="zT")

                if cut(10):
                    return
                # ---------- K: outN = z^T Wv^T + Sw (x) bv ----------------
                outp = zps_out[0:8, :]
                for dc in range(4):
                    nc.tensor.matmul(
                        outp,
                        lhsT=zsl[dc],
                        rhs=wvT[:, dc * 512 : (dc + 1) * 512],
                        start=(dc == 0),
                        stop=False,
                    )
                nc.tensor.matmul(
                    outp, lhsT=swS[:], rhs=bvr[:], start=False, stop=True,
                )
                outNs = work_pool.tile([8, 512], F32, tag="outNs")
                nc.scalar.activation(outNs[:], outp, AT.Copy)
                nc.gpsimd.dma_start(out_d[bass.ts(sb, SUPER), :], outNs[:])

            def body(_iv=None):
                load_consts()
                for sb in range(n_sb):
                    emit_superblock(sb)

            if repeat == 1:
                body()
            else:
                with tc.For_i(0, repeat, 1) as _iv:
                    body(_iv)

    nc.compile()
    return nc


def _prearrange_T(X):
    """[16384, 512] -> transposed load layout [sb*128+p, dc*512+r]."""
    return (
        X.T.reshape(4, 128, NSB, 512).transpose(2, 1, 0, 3).reshape(NSB * 128, 2048)
    ).copy()


def _prearrange_T8(X):
    """[16384, 512] -> DoubleRow transposed layout [sb*128+p, (d2, ko, r)]."""
    return (
        X.T.reshape(2, 2, 128, NSB, 512)
        .transpose(3, 2, 0, 1, 4)
        .reshape(NSB * 128, 2048)
    ).copy()


def _prearrange_N(X):
    """[16384, 512] -> natural load layout [sb*128+p, r*512+d]."""
    return (
        X.reshape(NSB, 4, 128, 512).transpose(0, 2, 1, 3).reshape(NSB * 128, 2048)
    ).copy()


def prep_in_maps(query, key, value, Wq, bq, Wk, bk, Wv, bv, Wr1, br1, Wr2, br2):
    s = np.float32(1.0 / np.sqrt(np.float32(D)))
    # scores = Xq (Wq^T Wk * s) Xk^T + 1 (Xk Wk^T bq * s)^T  (+ row consts,
    # which softmax ignores); the second term folds into Y as "+ 1 wkb^T".
    M = (Wq.T.astype(np.float32) @ Wk.astype(np.float32)) * s  # [d, e]
    wkb_full = (Wk.T.astype(np.float32) @ bq.astype(np.float32)) * s  # [e]
    if FP8_QK[0]:
        # mT8[p, d2, ko, e] = 64*M[d2*256+ko*128+p, e]
        mT = (
            (M * FP8_SCALE)
            .reshape(2, 2, 128, 512)
            .transpose(2, 0, 1, 3)
            .reshape(128, 2048)
            .astype(F8_NP)
            .copy()
        )
        wkb = (
            (wkb_full * FP8_SCALE).astype(np.float32).reshape(4, 128).T.copy()
        )
    else:
        mT = M.astype(BF16_NP).copy()
        wkb = wkb_full.astype(np.float32).reshape(4, 128).T.copy()  # [128, 4]
    wvT = Wv.T.astype(BF16_NP).copy()
    w1vT = (Wr1 @ Wv).T.astype(BF16_NP).copy()
    b1 = (br1 + Wr1 @ bv).astype(np.float32)
    b1c = np.concatenate([b1, b1]).reshape(128, 1).copy()
    wr2s = np.concatenate([Wr2[0], Wr2[0]]).astype(BF16_NP).reshape(128, 1).copy()
    bvr = bv.astype(BF16_NP).reshape(1, D).copy()
    ones = np.ones((128, 1), dtype=BF16_NP)
    # the hid2 zero blocks turn into prelu(b1) and leak C into every w
    slope = 0.0 if DBG_RELU[0] else LEAK
    pb1 = np.where(b1 >= 0, b1, slope * b1).astype(np.float32)
    C = float(pb1 @ Wr2[0].astype(np.float32))
    BR2_VAL[0] = float(br2[0]) - C

    in_maps = []
    for c in range(NCORES):
        sl = slice(c * BS_SH, (c + 1) * BS_SH)
        QK_NP = F8_NP if FP8_QK[0] else BF16_NP
        Xq = np.ascontiguousarray(query[sl]).reshape(NPAIR * NSHOT, D).astype(QK_NP)
        Xk = np.ascontiguousarray(key[sl]).reshape(NPAIR * NSHOT, D).astype(QK_NP)
        Xv = np.ascontiguousarray(value[sl]).reshape(NPAIR * NSHOT, D).astype(BF16_NP)
        pT = _prearrange_T8 if FP8_QK[0] else _prearrange_T
        in_maps.append(
            {
                "xqT": pT(Xq),
                "xkT": pT(Xk),
                "xvT": _prearrange_T(Xv),
                "xv": _prearrange_N(Xv),
                ("mT8" if FP8_QK[0] else "mT"): mT,
                "wvT": wvT,
                "w1vT": w1vT,
                "wkb": wkb,
                "wr2s": wr2s,
                "b1c": b1c,
                "bvr": bvr,
                "ones": ones,
            }
        )
    return in_maps


_nc_cache = {}


def kernel(**inputs):
    in_maps = prep_in_maps(**{k: np.asarray(v) for k, v in inputs.items()})
    key = ("k", 1, BR2_VAL[0], Z_VIA_XBAR[0], DBG_RELU[0], USE_LRELU[0], STAGE[0], FP8_QK[0])
    if key not in _nc_cache:
        _nc_cache[key] = build_nc(repeat=1)
    nc = _nc_cache[key]
    res = run_bass_kernel_spmd(nc, in_maps, core_ids=list(range(NCORES)))
    outs = [res.results[c]["out"].reshape(BS_SH, NWAY, D) for c in range(NCORES)]
    return np.concatenate(outs, axis=0).astype(np.float32)
